# revision 11
# baseline (speedup 1.0000x reference)
"""MultiHeadAttention (B=4, T=2048, C=1024, H=16, Dh=64) on 8 trn2 cores.

Sharding: 2 batch-groups x 4 head-groups. Core c handles batches
[2*(c//4), 2*(c//4)+1] and heads [4*(c%4) .. 4*(c%4)+3]. Each core
computes qkv projection for its heads, attention, and a partial
out-projection (its 256 head-dims of the 1024-dim contraction).
Host sums the 4 partials per batch-group.

All matmuls run in float32r (full PE rate, ~1.5e-4 rel err).
Attention uses transposed scores S^T[k,q] so the softmax needs no
transposes: exp via ACT (no max subtraction -- |S| < ~8 for these
magnitudes), the softmax denominator comes from a ones-column packed
next to V in the PV stationary, and the division is a broadcast
multiply folded into the d-major attention output.
"""

import json
import numpy as np

import concourse.bass as bass
import concourse.mybir as mybir
import concourse.tile as tile
from concourse.bass_utils import run_bass_kernel_spmd

F32 = mybir.dt.float32
F32R = mybir.dt.float32r

B, T, C = 4, 2048, 1024
H, DH = 16, 64
SCALE = DH ** -0.5
N_CORES = 8
BPC = 2            # batches per core
HPC = 4            # heads per core
TOK = BPC * T      # tokens per core (4096)
CK = C // 128      # contraction chunks (8)
TG = 512           # token group (moving N for qkv)
KC = T // 128      # key chunks per batch (16)
QT = T // 512      # q tiles per batch (4)

_MAX_WAITS = 1


def _split_multi_waits(d):
    """The axon-client walrus build rejects instructions carrying more
    than one sync-wait command. Move extra waits onto wait-only
    EventSemaphore instructions inserted immediately before, on the
    same engine."""
    n = 0
    for fn in d.get("functions", []):
        for blk in fn.get("blocks", []):
            out = []
            for inst in blk.get("instructions", []):
                si = inst.get("sync_info")
                waits = si.get("on_wait") if si else None
                if waits and len(waits) > _MAX_WAITS:
                    for w in waits[:-_MAX_WAITS]:
                        n += 1
                        out.append({
                            "engine": inst["engine"],
                            "ins": [], "outs": [],
                            "name": f"{inst['name']}_xw{n}",
                            "opcode": "EventSemaphore",
                            "debug": inst.get("debug", 0),
                            "sync_info": {"on_update": [], "on_wait": [w]},
                        })
                    si["on_wait"] = waits[-_MAX_WAITS:]
                out.append(inst)
            blk["instructions"] = out
    return d


def _patch_multiwait():
    if getattr(bass.Bass, "_multiwait_patched", False):
        return
    orig = bass.Bass.to_json_bytes

    def to_json_bytes(self, *a, **k):
        d = json.loads(orig(self, *a, **k))
        return json.dumps(_split_multi_waits(d)).encode()

    bass.Bass.to_json_bytes = to_json_bytes
    bass.Bass._multiwait_patched = True


def build_nc():
    nc = bass.Bass("TRN2", target_bir_lowering=False)
    xT = nc.dram_tensor("xT", [128, CK, TOK], F32R, kind="ExternalInput")
    w_qk = nc.dram_tensor("w_qk", [128, CK, 512], F32R, kind="ExternalInput")
    b_qk = nc.dram_tensor("b_qk", [128, 4], F32, kind="ExternalInput")
    w_v = nc.dram_tensor("w_v", [128, CK, 256], F32R, kind="ExternalInput")
    b_v = nc.dram_tensor("b_v", [128, 256], F32, kind="ExternalInput")
    w_out = nc.dram_tensor("w_out", [128, 2, 1024], F32R, kind="ExternalInput")
    b_out = nc.dram_tensor("b_out", [128, 1024], F32, kind="ExternalInput")
    y = nc.dram_tensor("y", [TOK, 1024], F32, kind="ExternalOutput")

    Exp = mybir.ActivationFunctionType.Exp
    Ln = mybir.ActivationFunctionType.Ln
    mult = mybir.AluOpType.mult
    add = mybir.AluOpType.add

    with tile.TileContext(nc) as tc:
        with (
            tc.tile_pool(name="wts", bufs=1) as wts,
            tc.tile_pool(name="xt", bufs=10) as xt_pool,
            tc.tile_pool(name="qkv", bufs=2) as qkv_pool,
            tc.tile_pool(name="pt", bufs=3) as pt_pool,
            tc.tile_pool(name="ot", bufs=2) as ot_pool,
            tc.tile_pool(name="ysb", bufs=3) as y_pool,
            tc.tile_pool(name="sm", bufs=3) as sm_pool,
            tc.tile_pool(name="ps", bufs=2, space="PSUM") as ps_pool,
            tc.tile_pool(name="ov", bufs=2, space="PSUM") as ov_pool,
            tc.tile_pool(name="stb", bufs=2, space="PSUM") as stb_pool,
        ):
            w_qk_sb = wts.tile([128, CK, 512], F32R)
            w_v_sb = wts.tile([128, CK, 256], F32R)
            w_out_sb = wts.tile([128, 2, 1024], F32R)
            b_qk_sb = wts.tile([128, 4], F32)
            b_v_sb = wts.tile([128, 256], F32)
            b_out_sb = wts.tile([128, 1024], F32)
            warm = wts.tile([1, 4], F32)
            nc.gpsimd.memset(warm[:], 1.0)
            nc.scalar.activation(warm[0:1, 0:1], warm[0:1, 2:3], Exp)
            nc.scalar.activation(warm[0:1, 1:2], warm[0:1, 3:4], Ln)
            for kc in range(CK):
                nc.sync.dma_start(w_qk_sb[:, kc, :], w_qk[:, kc, :])
                nc.sync.dma_start(w_v_sb[:, kc, :], w_v[:, kc, :])
            nc.sync.dma_start(w_out_sb[:], w_out[:])
            nc.sync.dma_start(b_qk_sb[:], b_qk[:])
            nc.sync.dma_start(b_v_sb[:], b_v[:])
            nc.sync.dma_start(b_out_sb[:], b_out[:])

            for b in range(BPC):
                # ---- qkv projection for batch b ----
                qT = [qkv_pool.tile([128, T], F32R, tag=f"qT{hp}", name=f"qT{hp}_{b}") for hp in range(2)]
                kT = [qkv_pool.tile([128, T], F32R, tag=f"kT{hp}", name=f"kT{hp}_{b}") for hp in range(2)]
                # v_sb[:, kc, h*65:h*65+64] = V k-major; col h*65+64 = ones
                v_sb = qkv_pool.tile([128, KC, 260], F32R, tag="v")
                v4 = v_sb[:].rearrange("p k (h c) -> p k h c", c=65)
                for h in range(HPC):
                    nc.gpsimd.memset(v4[:, :, h, 64:65].bitcast(F32), 1.0)

                for g in range(T // TG):
                    xts = []
                    for kc in range(CK):
                        xt = xt_pool.tile([128, TG], F32R, tag="xt")
                        nc.sync.dma_start(
                            xt[:], xT[:, kc, b * T + g * TG: b * T + (g + 1) * TG])
                        xts.append(xt)
                    # interleave d-major q^T/k^T m-chunks with token-major V
                    # t-chunks: the 256-row V matmuls alone leave the PE
                    # LDW-dominated and trip the HAM clock gate.
                    for m in range(4):
                        ps = ps_pool.tile([128, TG], F32, tag="ps")
                        for kc in range(CK):
                            nc.tensor.matmul(
                                ps[:], w_qk_sb[:, kc, m * 128:(m + 1) * 128],
                                xts[kc][:],
                                start=(kc == 0), stop=(kc == CK - 1))
                        dest = (qT if m < 2 else kT)[m % 2]
                        nc.vector.tensor_scalar_add(
                            dest[:, g * TG:(g + 1) * TG], ps[:], b_qk_sb[:, m:m + 1])
                        t = m
                        psv = ps_pool.tile([128, 256], F32, tag="ps")
                        for kc in range(CK):
                            nc.tensor.matmul(
                                psv[:], xts[kc][:, t * 128:(t + 1) * 128],
                                w_v_sb[:, kc, :],
                                start=(kc == 0), stop=(kc == CK - 1))
                        kchunk = g * (TG // 128) + t
                        nc.vector.tensor_tensor(
                            v4[:, kchunk, :, 0:64],
                            psv[:].rearrange("p (h c) -> p h c", c=64),
                            b_v_sb[:].rearrange("p (h c) -> p h c", c=64),
                            op=add)

                # ---- attention + out-projection for batch b ----
                for qt in range(QT):
                    outT = ot_pool.tile([128, 2, 512], F32R, tag="outT")
                    for h in range(HPC):
                        hp, hi = h // 2, h % 2
                        r0, r1 = hi * 64, (hi + 1) * 64
                        ov = ov_pool.tile([128, 512], F32, tag="ov")
                        for kg in range(KC // 2):
                            stb = stb_pool.tile([128, 1024], F32, tag="stb")
                            for j in range(2):
                                kc = kg * 2 + j
                                nc.tensor.matmul(
                                    stb[:, j * 512:(j + 1) * 512],
                                    kT[hp][r0:r1, kc * 128:(kc + 1) * 128],
                                    qT[hp][r0:r1, qt * 512:(qt + 1) * 512])
                            pt = pt_pool.tile([128, 1024], F32R, tag="pt")
                            nc.scalar.activation(pt[:], stb[:], Exp, scale=SCALE)
                            for j in range(2):
                                kc = kg * 2 + j
                                nc.tensor.matmul(
                                    ov[0:65, :], v_sb[:, kc, h * 65:h * 65 + 65],
                                    pt[:, j * 512:(j + 1) * 512],
                                    start=(kc == 0), stop=(kc == KC - 1))
                        rb = sm_pool.tile([64, 512], F32, tag="rb")
                        # 1/s = exp(-ln(s)) on ACT: same table set as the
                        # softmax exp, and ~5x faster than DVE reciprocal.
                        nc.scalar.activation(rb[0:1, :], ov[64:65, :], Ln)
                        nc.scalar.activation(rb[0:1, :], rb[0:1, :], Exp, scale=-1.0)
                        nc.vector.stream_shuffle(rb[0:32, :], rb[0:32, :], [0] * 32)
                        nc.vector.tensor_copy(rb[32:64, :], rb[0:32, :])
                        nc.vector.tensor_tensor(
                            outT[r0:r1, hp, :], ov[0:64, :], rb[:], op=mult)
                    for t2 in range(4):
                        for nb in range(2):
                            yp = ps_pool.tile([128, 512], F32, tag="ps")
                            for dc in range(2):
                                nc.tensor.matmul(
                                    yp[:], outT[:, dc, t2 * 128:(t2 + 1) * 128],
                                    w_out_sb[:, dc, nb * 512:(nb + 1) * 512],
                                    start=(dc == 0), stop=(dc == 1))
                            ysb = y_pool.tile([128, 512], F32, tag="ysb")
                            nc.vector.tensor_tensor(
                                ysb[:], yp[:], b_out_sb[:, nb * 512:(nb + 1) * 512],
                                op=add)
                            row = b * T + qt * 512 + t2 * 128
                            nc.sync.dma_start(
                                y[row:row + 128, nb * 512:(nb + 1) * 512], ysb[:])
    return nc


def _shard_inputs(x, w_qkv, b_qkv, w_out, b_out):
    in_maps = []
    for c in range(N_CORES):
        bg, hg = c // HPC, c % HPC
        heads = range(HPC * hg, HPC * hg + HPC)
        x2 = x[BPC * bg: BPC * bg + BPC].reshape(TOK, C)
        xT = np.ascontiguousarray(x2.reshape(TOK, CK, 128).transpose(2, 1, 0))

        # m-chunks: [q h0,h1], [q h2,h3], [k h0,h1], [k h2,h3]
        cols, bias = [], []
        for sec in (0, 1):          # q section, k section
            for pair in (0, 1):
                for h in (heads[2 * pair], heads[2 * pair + 1]):
                    sl = slice(sec * C + h * DH, sec * C + (h + 1) * DH)
                    cols.append(w_qkv[:, sl])
                    bias.append(b_qkv[sl])
        w_qk = np.concatenate(cols, axis=1)                     # [C, 512]
        w_qk = np.ascontiguousarray(w_qk.reshape(CK, 128, 512).transpose(1, 0, 2))
        b_qk = np.ascontiguousarray(
            np.concatenate(bias).reshape(4, 128).T)             # [128, 4]

        vcols = [w_qkv[:, 2 * C + h * DH: 2 * C + (h + 1) * DH] for h in heads]
        w_v = np.concatenate(vcols, axis=1)                     # [C, 256]
        w_v = np.ascontiguousarray(w_v.reshape(CK, 128, 256).transpose(1, 0, 2))
        b_v = np.concatenate([b_qkv[2 * C + h * DH: 2 * C + (h + 1) * DH]
                              for h in heads])
        b_v = np.ascontiguousarray(np.broadcast_to(b_v, (128, 256)))

        rows = [w_out[h * DH:(h + 1) * DH, :] for h in heads]
        w_o = np.concatenate(rows, axis=0)                      # [256, 1024]
        w_o = np.ascontiguousarray(w_o.reshape(2, 128, 1024).transpose(1, 0, 2))
        b_o = np.ascontiguousarray(
            np.broadcast_to(b_out / HPC, (128, 1024)).astype(np.float32))

        in_maps.append({
            "xT": xT.astype(np.float32),
            "w_qk": w_qk.astype(np.float32),
            "b_qk": b_qk.astype(np.float32),
            "w_v": w_v.astype(np.float32),
            "b_v": b_v.astype(np.float32),
            "w_out": w_o.astype(np.float32),
            "b_out": b_o,
        })
    return in_maps


def kernel(x, w_qkv, b_qkv, w_out, b_out, _trace=False, _nc_cache={}):
    _patch_multiwait()
    x = np.asarray(x, dtype=np.float32)
    w_qkv = np.asarray(w_qkv, dtype=np.float32)
    b_qkv = np.asarray(b_qkv, dtype=np.float32)
    w_out = np.asarray(w_out, dtype=np.float32)
    b_out = np.asarray(b_out, dtype=np.float32)

    if "nc" not in _nc_cache:
        _nc_cache["nc"] = build_nc()
    nc = _nc_cache["nc"]
    in_maps = _shard_inputs(x, w_qkv, b_qkv, w_out, b_out)
    res = run_bass_kernel_spmd(nc, in_maps, list(range(N_CORES)), trace=_trace)

    y = np.zeros((B, T, C), dtype=np.float32)
    for c in range(N_CORES):
        bg = c // HPC
        y[BPC * bg: BPC * bg + BPC] += res.results[c]["y"].reshape(BPC, T, C)
    if _trace:
        return y, res
    return y


# revision 13
# speedup vs baseline: 1.0339x; 1.0339x over previous
"""MultiHeadAttention (B=4, T=2048, C=1024, H=16, Dh=64) on 8 trn2 cores.

Sharding: 2 batch-groups x 4 head-groups. Core c handles batches
[2*(c//4), 2*(c//4)+1] and heads [4*(c%4) .. 4*(c%4)+3]. Each core
computes qkv projection for its heads, attention, and a partial
out-projection (its 256 head-dims of the 1024-dim contraction).
Host sums the 4 partials per batch-group.

All matmuls run in float32r (full PE rate, ~1.5e-4 rel err).
Attention uses transposed scores S^T[k,q] so the softmax needs no
transposes: exp via ACT (no max subtraction -- |S| < ~8 for these
magnitudes), the softmax denominator comes from a ones-column packed
next to V in the PV stationary, and the division is a broadcast
multiply folded into the d-major attention output.
"""

import json
import numpy as np

import concourse.bass as bass
import concourse.mybir as mybir
import concourse.tile as tile
from concourse.bass_utils import run_bass_kernel_spmd

F32 = mybir.dt.float32
F32R = mybir.dt.float32r

B, T, C = 4, 2048, 1024
H, DH = 16, 64
SCALE = DH ** -0.5
N_CORES = 8
BPC = 2            # batches per core
HPC = 4            # heads per core
TOK = BPC * T      # tokens per core (4096)
CK = C // 128      # contraction chunks (8)
TG = 512           # token group (moving N for qkv)
KC = T // 128      # key chunks per batch (16)
QT = T // 512      # q tiles per batch (4)

_MAX_WAITS = 1


def _split_multi_waits(d):
    """The axon-client walrus build rejects instructions carrying more
    than one sync-wait command. Move extra waits onto wait-only
    EventSemaphore instructions inserted immediately before, on the
    same engine."""
    n = 0
    for fn in d.get("functions", []):
        for blk in fn.get("blocks", []):
            out = []
            for inst in blk.get("instructions", []):
                si = inst.get("sync_info")
                waits = si.get("on_wait") if si else None
                if waits and len(waits) > _MAX_WAITS:
                    for w in waits[:-_MAX_WAITS]:
                        n += 1
                        out.append({
                            "engine": inst["engine"],
                            "ins": [], "outs": [],
                            "name": f"{inst['name']}_xw{n}",
                            "opcode": "EventSemaphore",
                            "debug": inst.get("debug", 0),
                            "sync_info": {"on_update": [], "on_wait": [w]},
                        })
                    si["on_wait"] = waits[-_MAX_WAITS:]
                out.append(inst)
            blk["instructions"] = out
    return d


def _patch_multiwait():
    if getattr(bass.Bass, "_multiwait_patched", False):
        return
    orig = bass.Bass.to_json_bytes

    def to_json_bytes(self, *a, **k):
        d = json.loads(orig(self, *a, **k))
        return json.dumps(_split_multi_waits(d)).encode()

    bass.Bass.to_json_bytes = to_json_bytes
    bass.Bass._multiwait_patched = True


def build_nc():
    nc = bass.Bass("TRN2", target_bir_lowering=False)
    xT = nc.dram_tensor("xT", [128, CK, TOK], F32R, kind="ExternalInput")
    w_qk = nc.dram_tensor("w_qk", [128, CK, 512], F32R, kind="ExternalInput")
    b_qk = nc.dram_tensor("b_qk", [128, 4], F32, kind="ExternalInput")
    w_v = nc.dram_tensor("w_v", [128, CK, 256], F32R, kind="ExternalInput")
    b_v = nc.dram_tensor("b_v", [128, 256], F32, kind="ExternalInput")
    w_out = nc.dram_tensor("w_out", [128, 2, 1024], F32R, kind="ExternalInput")
    b_out = nc.dram_tensor("b_out", [128, 1024], F32, kind="ExternalInput")
    y = nc.dram_tensor("y", [TOK, 1024], F32, kind="ExternalOutput")

    Exp = mybir.ActivationFunctionType.Exp
    Ln = mybir.ActivationFunctionType.Ln
    mult = mybir.AluOpType.mult
    add = mybir.AluOpType.add

    with tile.TileContext(nc) as tc:
        with (
            tc.tile_pool(name="wts", bufs=1) as wts,
            tc.tile_pool(name="xt", bufs=10) as xt_pool,
            tc.tile_pool(name="qkv", bufs=2) as qkv_pool,
            tc.tile_pool(name="pt", bufs=3) as pt_pool,
            tc.tile_pool(name="ot", bufs=2) as ot_pool,
            tc.tile_pool(name="ysb", bufs=3) as y_pool,
            tc.tile_pool(name="sm", bufs=3) as sm_pool,
            tc.tile_pool(name="ps", bufs=2, space="PSUM") as ps_pool,
            tc.tile_pool(name="ov", bufs=2, space="PSUM") as ov_pool,
            tc.tile_pool(name="stb", bufs=2, space="PSUM") as stb_pool,
        ):
            w_qk_sb = wts.tile([128, CK, 512], F32R)
            w_v_sb = wts.tile([128, CK, 256], F32R)
            w_out_sb = wts.tile([128, 2, 1024], F32R)
            b_qk_sb = wts.tile([128, 4], F32)
            b_v_sb = wts.tile([128, 256], F32)
            b_out_sb = wts.tile([128, 1024], F32)
            warm = wts.tile([1, 512], F32R)
            nc.gpsimd.memset(warm[:].bitcast(F32), 1.0)
            warma = wts.tile([1, 4], F32)
            nc.gpsimd.memset(warma[:], 1.0)
            nc.scalar.activation(warma[0:1, 0:1], warma[0:1, 2:3], Exp)
            nc.scalar.activation(warma[0:1, 1:2], warma[0:1, 3:4], Ln)
            # keep the PE busy while the first DMAs land so the HAM
            # clock gate ramps to full rate before real work arrives
            wps = ps_pool.tile([1, 512], F32, tag="ps", name="warm_ps")
            for _ in range(24):
                nc.tensor.matmul(wps[:], warm[0:1, 0:1], warm[:])
            for kc in range(CK):
                nc.sync.dma_start(w_qk_sb[:, kc, :], w_qk[:, kc, :])
                nc.sync.dma_start(w_v_sb[:, kc, :], w_v[:, kc, :])
            nc.sync.dma_start(w_out_sb[:], w_out[:])
            nc.sync.dma_start(b_qk_sb[:], b_qk[:])
            nc.sync.dma_start(b_v_sb[:], b_v[:])
            nc.sync.dma_start(b_out_sb[:], b_out[:])

            for b in range(BPC):
                # ---- qkv projection for batch b ----
                qT = [qkv_pool.tile([128, T], F32R, tag=f"qT{hp}", name=f"qT{hp}_{b}") for hp in range(2)]
                kT = [qkv_pool.tile([128, T], F32R, tag=f"kT{hp}", name=f"kT{hp}_{b}") for hp in range(2)]
                # v_sb[:, kc, h*65:h*65+64] = V k-major; col h*65+64 = ones
                v_sb = qkv_pool.tile([128, KC, 260], F32R, tag="v")
                v4 = v_sb[:].rearrange("p k (h c) -> p k h c", c=65)
                for h in range(HPC):
                    nc.gpsimd.memset(v4[:, :, h, 64:65].bitcast(F32), 1.0)

                for g in range(T // TG):
                    xts = []
                    for kc in range(CK):
                        xt = xt_pool.tile([128, TG], F32R, tag="xt")
                        nc.sync.dma_start(
                            xt[:], xT[:, kc, b * T + g * TG: b * T + (g + 1) * TG])
                        xts.append(xt)
                    # interleave d-major q^T/k^T m-chunks with token-major V
                    # t-chunks: the 256-row V matmuls alone leave the PE
                    # LDW-dominated and trip the HAM clock gate.
                    for m in range(4):
                        ps = ps_pool.tile([128, TG], F32, tag="ps")
                        for kc in range(CK):
                            nc.tensor.matmul(
                                ps[:], w_qk_sb[:, kc, m * 128:(m + 1) * 128],
                                xts[kc][:],
                                start=(kc == 0), stop=(kc == CK - 1))
                        dest = (qT if m < 2 else kT)[m % 2]
                        nc.vector.tensor_scalar_add(
                            dest[:, g * TG:(g + 1) * TG], ps[:], b_qk_sb[:, m:m + 1])
                        t = m
                        psv = ps_pool.tile([128, 256], F32, tag="ps")
                        for kc in range(CK):
                            nc.tensor.matmul(
                                psv[:], xts[kc][:, t * 128:(t + 1) * 128],
                                w_v_sb[:, kc, :],
                                start=(kc == 0), stop=(kc == CK - 1))
                        kchunk = g * (TG // 128) + t
                        nc.vector.tensor_tensor(
                            v4[:, kchunk, :, 0:64],
                            psv[:].rearrange("p (h c) -> p h c", c=64),
                            b_v_sb[:].rearrange("p (h c) -> p h c", c=64),
                            op=add)

                # ---- attention + out-projection for batch b ----
                for qt in range(QT):
                    outT = ot_pool.tile([128, 2, 512], F32R, tag="outT")
                    for h in range(HPC):
                        hp, hi = h // 2, h % 2
                        r0, r1 = hi * 64, (hi + 1) * 64
                        ov = ov_pool.tile([128, 512], F32, tag="ov")
                        for kg in range(KC // 2):
                            stb = stb_pool.tile([128, 1024], F32, tag="stb")
                            for j in range(2):
                                kc = kg * 2 + j
                                nc.tensor.matmul(
                                    stb[:, j * 512:(j + 1) * 512],
                                    kT[hp][r0:r1, kc * 128:(kc + 1) * 128],
                                    qT[hp][r0:r1, qt * 512:(qt + 1) * 512])
                            pt = pt_pool.tile([128, 1024], F32R, tag="pt")
                            nc.scalar.activation(pt[:], stb[:], Exp, scale=SCALE)
                            for j in range(2):
                                kc = kg * 2 + j
                                nc.tensor.matmul(
                                    ov[0:65, :], v_sb[:, kc, h * 65:h * 65 + 65],
                                    pt[:, j * 512:(j + 1) * 512],
                                    start=(kc == 0), stop=(kc == KC - 1))
                        rb = sm_pool.tile([64, 512], F32, tag="rb")
                        # 1/s = exp(-ln(s)) on ACT: same table set as the
                        # softmax exp, and ~5x faster than DVE reciprocal.
                        nc.scalar.activation(rb[0:1, :], ov[64:65, :], Ln)
                        nc.scalar.activation(rb[0:1, :], rb[0:1, :], Exp, scale=-1.0)
                        nc.vector.stream_shuffle(rb[0:32, :], rb[0:32, :], [0] * 32)
                        nc.vector.tensor_copy(rb[32:64, :], rb[0:32, :])
                        nc.vector.tensor_tensor(
                            outT[r0:r1, hp, :], ov[0:64, :], rb[:], op=mult)
                    for t2 in range(4):
                        for nb in range(2):
                            yp = ps_pool.tile([128, 512], F32, tag="ps")
                            for dc in range(2):
                                nc.tensor.matmul(
                                    yp[:], outT[:, dc, t2 * 128:(t2 + 1) * 128],
                                    w_out_sb[:, dc, nb * 512:(nb + 1) * 512],
                                    start=(dc == 0), stop=(dc == 1))
                            ysb = y_pool.tile([128, 512], F32, tag="ysb")
                            nc.vector.tensor_tensor(
                                ysb[:], yp[:], b_out_sb[:, nb * 512:(nb + 1) * 512],
                                op=add)
                            row = b * T + qt * 512 + t2 * 128
                            nc.sync.dma_start(
                                y[row:row + 128, nb * 512:(nb + 1) * 512], ysb[:])
    return nc


def _shard_inputs(x, w_qkv, b_qkv, w_out, b_out):
    in_maps = []
    for c in range(N_CORES):
        bg, hg = c // HPC, c % HPC
        heads = range(HPC * hg, HPC * hg + HPC)
        x2 = x[BPC * bg: BPC * bg + BPC].reshape(TOK, C)
        xT = np.ascontiguousarray(x2.reshape(TOK, CK, 128).transpose(2, 1, 0))

        # m-chunks: [q h0,h1], [q h2,h3], [k h0,h1], [k h2,h3]
        cols, bias = [], []
        for sec in (0, 1):          # q section, k section
            for pair in (0, 1):
                for h in (heads[2 * pair], heads[2 * pair + 1]):
                    sl = slice(sec * C + h * DH, sec * C + (h + 1) * DH)
                    cols.append(w_qkv[:, sl])
                    bias.append(b_qkv[sl])
        w_qk = np.concatenate(cols, axis=1)                     # [C, 512]
        w_qk = np.ascontiguousarray(w_qk.reshape(CK, 128, 512).transpose(1, 0, 2))
        b_qk = np.ascontiguousarray(
            np.concatenate(bias).reshape(4, 128).T)             # [128, 4]

        vcols = [w_qkv[:, 2 * C + h * DH: 2 * C + (h + 1) * DH] for h in heads]
        w_v = np.concatenate(vcols, axis=1)                     # [C, 256]
        w_v = np.ascontiguousarray(w_v.reshape(CK, 128, 256).transpose(1, 0, 2))
        b_v = np.concatenate([b_qkv[2 * C + h * DH: 2 * C + (h + 1) * DH]
                              for h in heads])
        b_v = np.ascontiguousarray(np.broadcast_to(b_v, (128, 256)))

        rows = [w_out[h * DH:(h + 1) * DH, :] for h in heads]
        w_o = np.concatenate(rows, axis=0)                      # [256, 1024]
        w_o = np.ascontiguousarray(w_o.reshape(2, 128, 1024).transpose(1, 0, 2))
        b_o = np.ascontiguousarray(
            np.broadcast_to(b_out / HPC, (128, 1024)).astype(np.float32))

        in_maps.append({
            "xT": xT.astype(np.float32),
            "w_qk": w_qk.astype(np.float32),
            "b_qk": b_qk.astype(np.float32),
            "w_v": w_v.astype(np.float32),
            "b_v": b_v.astype(np.float32),
            "w_out": w_o.astype(np.float32),
            "b_out": b_o,
        })
    return in_maps


def kernel(x, w_qkv, b_qkv, w_out, b_out, _trace=False, _nc_cache={}):
    _patch_multiwait()
    x = np.asarray(x, dtype=np.float32)
    w_qkv = np.asarray(w_qkv, dtype=np.float32)
    b_qkv = np.asarray(b_qkv, dtype=np.float32)
    w_out = np.asarray(w_out, dtype=np.float32)
    b_out = np.asarray(b_out, dtype=np.float32)

    if "nc" not in _nc_cache:
        _nc_cache["nc"] = build_nc()
    nc = _nc_cache["nc"]
    in_maps = _shard_inputs(x, w_qkv, b_qkv, w_out, b_out)
    res = run_bass_kernel_spmd(nc, in_maps, list(range(N_CORES)), trace=_trace)

    y = np.zeros((B, T, C), dtype=np.float32)
    for c in range(N_CORES):
        bg = c // HPC
        y[BPC * bg: BPC * bg + BPC] += res.results[c]["y"].reshape(BPC, T, C)
    if _trace:
        return y, res
    return y
